# revision 1
# baseline (speedup 1.0000x reference)
"""Trainium2 Bass kernel for EvalMemoryReader (retrieval_knn).

Distributed plan (8 NeuronCores, memory axis THW sharded -> 1 frame/core):
  A. stage-1 fp32 matmul (layout A: mem rows x queries) -> per-row argmax
     via DVE max8+find-index -> gaussian center (y,x) per memory row.
  B. stage-2 fp32 matmul (layout B: queries x mem rows) with augmented
     channels folding the gaussian + per-row constants into the contraction
     -> selection scores s_hat.  Per-16 segment maxes -> AllToAll (query
     sharded) -> rank-51-of-segmaxes threshold t + column max via 7 rounds
     of max8/match_replace -> AllGather.  Local survivor counts + 8 smallest
     survivors per query (masked max8) -> AllGather -> exact v50/v51 of the
     global top-50 boundary -> tau = midpoint.
  C. fp32 matmul (layout A) recomputes scores minus column max; mask at tau
     (exact top-50) * exp -> bf16 weights; bf16 readout matmul with a ones
     row appended for the softmax normalizer; AllReduce; normalize.

kernel() takes FULL inputs, shards host-side, runs SPMD on cores 0-7.
"""

import math
import os

import ml_dtypes
import numpy as np

import concourse.bass as bass
import concourse.bacc as bacc
import concourse.mybir as mybir
from concourse.tile import TileContext

ND = 8
CK, CV, T, H, W = 64, 512, 8, 32, 56
HW = H * W              # 1792 queries
THW = T * HW            # 14336 memory locations
ML = THW // ND          # 1792 memory rows per core (exactly one frame)
NCH = HW // 128         # 14 chunks of 128 (query or mem rows)
NB = 4                  # 448-wide free-dim chunks per 1792
NW = HW // NB           # 448
SEG = 16
NSEG = ML // SEG        # 112 segments per core
NQ = HW // ND           # 224 queries per core for threshold extraction
GD = 2.0 * 5.6 * 5.6    # 62.72
CG = math.sqrt(2.0 / GD)
BIG = 1.0e30
NEG = -1.0e30
MCV = CV // 128         # 4 output chunks

F32 = mybir.dt.float32
BF16 = mybir.dt.bfloat16
U32 = mybir.dt.uint32
ALU = mybir.AluOpType
ACT = mybir.ActivationFunctionType
AX = mybir.AxisListType


class _Trunc(Exception):
    pass


def _build():
    nc = bacc.Bacc(num_devices=ND)

    msa_d = nc.dram_tensor("msa", [65, ML], F32, kind="ExternalInput")
    msb_d = nc.dram_tensor("msb", [68, ML], F32, kind="ExternalInput")
    q1_d = nc.dram_tensor("q1", [65, HW], F32, kind="ExternalInput")
    q2_d = nc.dram_tensor("q2", [67, HW], F32, kind="ExternalInput")
    q2c_d = nc.dram_tensor("q2c", [68, HW], F32, kind="ExternalInput")
    a8t_d = nc.dram_tensor("a8t", [128, NCH], F32, kind="ExternalInput")
    mvt_d = nc.dram_tensor("mvt", [128, NCH * CV], BF16, kind="ExternalInput")
    out_d = nc.dram_tensor("out", [CV // ND, HW], F32, kind="ExternalOutput")

    iota16_c = nc.inline_tensor(
        np.broadcast_to(np.arange(16, dtype=np.float32), (128, 16)).copy(),
        name="iota16")
    ones_1x128_c = nc.inline_tensor(np.ones((1, 128), np.float32), name="o1x128")
    ones_128x1_c = nc.inline_tensor(
        np.ones((128, 1), np.float32).astype(ml_dtypes.bfloat16), name="o128x1")
    thr56_c = nc.inline_tensor(
        np.broadcast_to(np.arange(1, H, dtype=np.float32) * W, (128, H - 1))
        .copy(), name="thr56")

    # collective bounce buffers
    segmax_l = nc.dram_tensor("segmax_l", [HW, NSEG], F32)
    segmax_x = nc.dram_tensor("segmax_x", [HW, NSEG], F32)
    tstats_l = nc.dram_tensor("tstats_l", [NQ, 2], F32)
    tstats_g = nc.dram_tensor("tstats_g", [HW, 2], F32, addr_space="Shared")
    stats_l = nc.dram_tensor("stats_l", [HW, 9], F32)
    stats_g = nc.dram_tensor("stats_g", [ND, HW, 9], F32, addr_space="Shared")
    scr = [nc.dram_tensor(f"scr{i}", [HW], F32) for i in range(6)]
    ro_l = nc.dram_tensor("ro_l", [CV + 1, HW], F32)
    rs_l = nc.dram_tensor("rs_l", [CV // ND, HW], F32)
    nrm_g = nc.dram_tensor("nrm_g", [1, HW], F32, addr_space="Shared")

    groups = [list(range(ND))]

    from contextlib import ExitStack
    with TileContext(nc) as tc, ExitStack() as es:
        try:
            POOL_E = mybir.EngineType.Pool
            cpool = es.enter_context(tc.tile_pool(name="consts", bufs=1))
            def cload(ap, name):
                return cpool.tile_from(ap, force_copy=True, name=name,
                                       forced_dma_engine=POOL_E)
            msa = cload(msa_d[:], "msa_t")
            msb = cload(msb_d[:], "msb_t")
            q1 = cload(q1_d[:], "q1_t")
            q2 = cload(q2_d[:], "q2_t")
            q2c = cload(q2c_d[:], "q2c_t")
            a8t = cload(a8t_d[:], "a8t_t")
            iota16 = cload(iota16_c[:], "iota16_t")
            ones_r = cload(ones_1x128_c[:], "ones_r_t")
            ones_cb = cload(ones_128x1_c[:], "ones_cb_t")
            thr56 = cload(thr56_c[:], "thr56_t")

            spool = es.enter_context(tc.tile_pool(name="smalls", bufs=1))
            ycg_t = spool.tile([128, NCH], F32)
            xcg_t = spool.tile([128, NCH], F32)
            alp_t = spool.tile([128, NCH], F32)
            t_all = spool.tile([128, NCH], F32)
            mx_all = spool.tile([128, NCH], F32)
            cnt_t = spool.tile([128, NCH], F32)
            mins_t = spool.tile([128, NCH, 8], F32)
            tauc_t = spool.tile([128, NCH], F32)
            tcb = spool.tile([128, NB, NW], F32)

            # ---------------- phase A: argmax per memory row ----------------
            with tc.tile_pool(name="psA", bufs=2, space="PSUM") as psA, \
                 tc.tile_pool(name="wkA", bufs=3) as wkA:
                for m in range(NCH):
                    ps = psA.tile([128, NB, 512], F32)
                    for j in range(NB):
                        nc.tensor.matmul(
                            ps[:, j, :NW],
                            lhsT=msa[:, m * 128:(m + 1) * 128],
                            rhs=q1[:, j * NW:(j + 1) * NW],
                            start=True, stop=True)
                    u = wkA.tile([128, HW], F32, tag="u")
                    nc.scalar.activation(
                        u.rearrange("p (j n) -> p j n", n=NW), ps[:, :, :NW], ACT.Copy)
                    m8 = wkA.tile([128, 8], F32, tag="m8")
                    i8 = wkA.tile([128, 8], U32, tag="i8")
                    nc.vector.max(m8, u)
                    nc.vector.max_index(i8, m8, u)
                    idxf = wkA.tile([128, 1], F32, tag="idxf")
                    nc.vector.tensor_copy(idxf, i8[:, 0:1])
                    xm = wkA.tile([128, 1], F32, tag="xm")
                    ym = wkA.tile([128, 1], F32, tag="ym")
                    jnk = wkA.tile([128, H - 1], F32, tag="jnk")
                    # y = #{k in 1..31 : k*W <= idx} = idx // W  (no mod/div on DVE)
                    nc.vector.tensor_scalar(jnk, thr56, idxf, None, op0=ALU.is_le,
                                            op1=ALU.add, accum_out=ym)
                    # x = idx - W*y
                    nc.vector.scalar_tensor_tensor(xm, ym, -float(W), idxf,
                                                   op0=ALU.mult, op1=ALU.add)
                    # scales/squares on ACT: keeps them off the DVE stream
                    # that the max8/max_index chain is saturating
                    nc.scalar.activation(ycg_t[:, m:m + 1], ym, ACT.Copy,
                                         scale=CG)
                    nc.scalar.activation(xcg_t[:, m:m + 1], xm, ACT.Copy,
                                         scale=CG)
                    # alpha = a8 + (y^2 + x^2)/GD ; note (y*cg)^2+(x*cg)^2 = 2(y^2+x^2)/GD
                    ysq = wkA.tile([128, 1], F32, tag="ysq")
                    nc.vector.tensor_mul(ysq, ycg_t[:, m:m + 1], ycg_t[:, m:m + 1])
                    xsq = wkA.tile([128, 1], F32, tag="xsq")
                    nc.vector.tensor_mul(xsq, xcg_t[:, m:m + 1], xcg_t[:, m:m + 1])
                    ssum = wkA.tile([128, 1], F32, tag="ssum")
                    nc.vector.tensor_add(ssum, ysq, xsq)
                    hsum = wkA.tile([128, 1], F32, tag="hsum")
                    nc.scalar.activation(hsum, ssum, ACT.Copy, scale=0.5)
                    nc.vector.tensor_add(alp_t[:, m:m + 1], hsum, a8t[:, m:m + 1])
                    # stream this chunk's gaussian channels into msb now so
                    # stage-2 matmuls can start before the argmax loop finishes
                    for row, srct in ((64, ycg_t), (65, xcg_t), (66, alp_t)):
                        nc.sync.dma_start(
                            out=msb[row:row + 1, m * 128:(m + 1) * 128],
                            in_=srct[:, m:m + 1])

            # scatter per-row channels into msb rows 64..66 (partition -> free),
            # bounced through DRAM because SBUF<->SBUF transpose APs don't balance
            def part_to_row(scratch, row_ap, tile_ap):
                nc.sync.dma_start(
                    out=scratch[:].rearrange("(m q) -> q m", q=128), in_=tile_ap)
                nc.sync.dma_start(out=row_ap, in_=scratch[:])


            PH = int(os.environ.get("KPHASE", "99"))
            if PH < 2:
                raise _Trunc()
            # ---------------- phase B: scores + top-50 boundary ----------------
            with tc.tile_pool(name="sBpool", bufs=1) as sBpool:
                sB = sBpool.tile([128, NCH, ML], F32)
                with tc.tile_pool(name="psB", bufs=2, space="PSUM") as psB, \
                     tc.tile_pool(name="wkB", bufs=3) as wkB:
                    for m in range(NCH):
                        ps = psB.tile([128, NB, 512], F32)
                        for j in range(NB):
                            nc.tensor.matmul(
                                ps[:, j, :NW],
                                lhsT=q2[:, m * 128:(m + 1) * 128],
                                rhs=msb[0:67, j * NW:(j + 1) * NW],
                                start=True, stop=True)
                        nc.scalar.activation(
                            sB[:, m, :].rearrange("p (j n) -> p j n", n=NW),
                            ps[:, :, :NW], ACT.Copy)
                        seg = wkB.tile([128, NSEG], F32, tag="seg")
                        nc.vector.tensor_reduce(
                            seg, sB[:, m, :].rearrange("p (s k) -> p s k", k=SEG),
                            axis=AX.X, op=ALU.max)
                        nc.sync.dma_start(out=segmax_l[m * 128:(m + 1) * 128, :],
                                          in_=seg[:])

                if PH < 3:
                    raise _Trunc()
                nc.gpsimd.collective_compute(
                    "AllToAll", ALU.bypass, replica_groups=groups,
                    ins=[segmax_l[:]], outs=[segmax_x[:]])

                # rank-51 of global segmaxes + column max, for my 224 queries
                with tc.tile_pool(name="wkT", bufs=2) as wkT:
                    for ci, (p0, pc) in enumerate(((0, 128), (128, 96))):
                        ext = wkT.tile([pc, ND * NSEG], F32, tag=f"ext{ci}")
                        src = segmax_x[:].rearrange("(d p) s -> p d s", d=ND)
                        nc.sync.dma_start(
                            out=ext.rearrange("p (d s) -> p d s", d=ND),
                            in_=src[p0:p0 + pc, :, :])
                        m8 = wkT.tile([pc, 8], F32, tag=f"m8{ci}")
                        mxc = wkT.tile([pc, 1], F32, tag=f"mx{ci}")
                        for r in range(7):
                            nc.vector.max(m8, ext)
                            if r == 0:
                                nc.vector.tensor_copy(mxc, m8[:, 0:1])
                            if r < 6:
                                nc.vector.match_replace(ext, m8, ext, NEG)
                        nc.sync.dma_start(out=tstats_l[p0:p0 + pc, 0:1],
                                          in_=m8[:, 2:3])
                        nc.sync.dma_start(out=tstats_l[p0:p0 + pc, 1:2], in_=mxc[:])

                nc.gpsimd.collective_compute(
                    "AllGather", ALU.bypass, replica_groups=groups,
                    ins=[tstats_l[:]], outs=[tstats_g[:]])

                nc.sync.dma_start(
                    out=t_all[:],
                    in_=tstats_g[:, 0:1].rearrange("(m q) s -> q (m s)", q=128))
                nc.sync.dma_start(
                    out=mx_all[:],
                    in_=tstats_g[:, 1:2].rearrange("(m q) s -> q (m s)", q=128))

                if PH < 4:
                    raise _Trunc()
                # local survivor count + 8 smallest survivors per query
                with tc.tile_pool(name="wkC", bufs=3) as wkC:
                    for m in range(NCH):
                        msk = wkC.tile([128, ML], F32, tag="msk")
                        nc.vector.tensor_scalar(
                            msk, sB[:, m, :], t_all[:, m:m + 1], None,
                            op0=ALU.is_lt, op1=ALU.add,
                            accum_out=cnt_t[:, m:m + 1])
                        r = wkC.tile([128, ML], F32, tag="r")
                        nc.vector.scalar_tensor_tensor(
                            r, msk, -BIG, sB[:, m, :], op0=ALU.mult, op1=ALU.subtract)
                        mn = wkC.tile([128, 8], F32, tag="mn")
                        nc.vector.max(mn, r)
                        nc.vector.tensor_scalar(mins_t[:, m, :], mn, -1.0, None,
                                                op0=ALU.mult)
                        nc.sync.dma_start(out=stats_l[m * 128:(m + 1) * 128, 0:1],
                                          in_=cnt_t[:, m:m + 1])
                        nc.sync.dma_start(out=stats_l[m * 128:(m + 1) * 128, 1:9],
                                          in_=mins_t[:, m, :])

            nc.gpsimd.collective_compute(
                "AllGather", ALU.bypass, replica_groups=groups,
                ins=[stats_l[:]], outs=[stats_g[:]])

            if PH < 5:
                raise _Trunc()
            # global boundary: v50/v51 -> tau, redundant on every core
            with tc.tile_pool(name="wkD", bufs=3) as wkD:
                for m in range(NCH):
                    mins64 = wkD.tile([128, ND, 8], F32, tag="m64")
                    nc.sync.dma_start(
                        out=mins64,
                        in_=stats_g[:, m * 128:(m + 1) * 128, 1:9]
                        .rearrange("d p s -> p d s"))
                    cnt8 = wkD.tile([128, ND], F32, tag="c8")
                    nc.sync.dma_start(
                        out=cnt8,
                        in_=stats_g[:, m * 128:(m + 1) * 128, 0:1]
                        .rearrange("d p s -> p (d s)"))
                    cl = wkD.tile([128, 1], F32, tag="cl")
                    nc.vector.tensor_reduce(cl, cnt8, axis=AX.X, op=ALU.add)
                    e = wkD.tile([128, 1], F32, tag="e")
                    # e = (THW - cnt_lt_total) - 50
                    nc.vector.tensor_scalar(e, cl, -1.0, float(THW - 50),
                                            op0=ALU.mult, op1=ALU.add)
                    neg64 = wkD.tile([128, ND * 8], F32, tag="n64")
                    nc.vector.tensor_scalar(
                        neg64, mins64.rearrange("p d s -> p (d s)"), -1.0, None,
                        op0=ALU.mult)
                    asc16 = wkD.tile([128, 16], F32, tag="a16")
                    a8a = wkD.tile([128, 8], F32, tag="a8a")
                    nc.vector.max(a8a, neg64)
                    nc.vector.tensor_scalar(asc16[:, 0:8], a8a, -1.0, None,
                                            op0=ALU.mult)
                    nc.vector.match_replace(neg64, a8a, neg64, NEG)
                    nc.vector.max(a8a, neg64)
                    nc.vector.tensor_scalar(asc16[:, 8:16], a8a, -1.0, None,
                                            op0=ALU.mult)
                    v50 = wkD.tile([128, 1], F32, tag="v50")
                    v51 = wkD.tile([128, 1], F32, tag="v51")
                    mk16 = wkD.tile([128, 16], F32, tag="mk16")
                    junk = wkD.tile([128, 16], F32, tag="junk")
                    nc.vector.tensor_scalar(mk16, iota16, e, None, op0=ALU.is_equal)
                    nc.vector.scalar_tensor_tensor(
                        junk, asc16, 1.0, mk16, op0=ALU.mult, op1=ALU.mult,
                        accum_out=v50)
                    em1 = wkD.tile([128, 1], F32, tag="em1")
                    nc.vector.tensor_scalar(em1, e, 1.0, None, op0=ALU.subtract)
                    nc.vector.tensor_scalar(mk16, iota16, em1, None, op0=ALU.is_equal)
                    nc.vector.scalar_tensor_tensor(
                        junk, asc16, 1.0, mk16, op0=ALU.mult, op1=ALU.mult,
                        accum_out=v51)
                    tau = wkD.tile([128, 1], F32, tag="tau")
                    nc.vector.tensor_add(tau, v50, v51)
                    nc.vector.tensor_scalar(tau, tau, 0.5, None, op0=ALU.mult)
                    nc.vector.tensor_sub(tauc_t[:, m:m + 1], tau, mx_all[:, m:m + 1])

            # move per-query scalars into rows; broadcast tau' across partitions
            part_to_row(scr[3], q2c[67:68, :], mx_all[:])
            with tc.tile_pool(name="wkE", bufs=1) as wkE, \
                 tc.tile_pool(name="psE", bufs=1, space="PSUM") as psE:
                taucrow = wkE.tile([1, HW], F32)
                part_to_row(scr[4], taucrow[:], tauc_t[:])
                psb = psE.tile([128, NB, 512], F32)
                for j in range(NB):
                    nc.tensor.matmul(psb[:, j, :NW], lhsT=ones_r,
                                     rhs=taucrow[:, j * NW:(j + 1) * NW],
                                     start=True, stop=True)
                nc.scalar.activation(tcb, psb[:, :, :NW], ACT.Copy)

            if PH < 6:
                raise _Trunc()
            # ---------------- phase C: weights + readout ----------------
            with tc.tile_pool(name="Wpool", bufs=1) as Wpool:
                Wt = Wpool.tile([128, NCH, ML], BF16)
                with tc.tile_pool(name="psC", bufs=2, space="PSUM") as psC, \
                     tc.tile_pool(name="wkF", bufs=3) as wkF:
                    for k in range(NCH):
                        ps = psC.tile([128, NB, 512], F32)
                        for j in range(NB):
                            nc.tensor.matmul(
                                ps[:, j, :NW],
                                lhsT=msb[:, k * 128:(k + 1) * 128],
                                rhs=q2c[:, j * NW:(j + 1) * NW],
                                start=True, stop=True)
                        mkt = wkF.tile([128, NB, NW], BF16, tag="mkt")
                        nc.vector.tensor_tensor(out=mkt, in0=ps[:, :, :NW], in1=tcb,
                                                op=ALU.is_ge)
                        ex = wkF.tile([128, NB, NW], BF16, tag="ex")
                        nc.scalar.activation(ex, ps[:, :, :NW], ACT.Exp)
                        nc.vector.tensor_mul(
                            Wt[:, k, :].rearrange("p (j n) -> p j n", n=NW), mkt, ex)

                if PH < 7:
                    raise _Trunc()
                with tc.tile_pool(name="mvpool", bufs=1) as mvpool, \
                     tc.tile_pool(name="wkO", bufs=2) as wkO:
                    mvt = mvpool.tile_from(mvt_d[:], force_copy=True,
                                           forced_dma_engine=POOL_E)
                    mvt3 = mvt.rearrange("p (k c) -> p k c", c=CV)
                    # norm row first in a short-lived PSUM pool so the value
                    # chunks below can double-buffer their PSUM tiles
                    with tc.tile_pool(name="psN", bufs=1, space="PSUM") as psN:
                        pn = psN.tile([1, NB, 512], F32)
                        for k in range(NCH):
                            for j in range(NB):
                                nc.tensor.matmul(
                                    pn[:, j, :NW], lhsT=ones_cb,
                                    rhs=Wt[:, k, j * NW:(j + 1) * NW],
                                    start=(k == 0), stop=(k == NCH - 1))
                        nb_ = wkO.tile([1, NB, NW], F32, tag="nb")
                        nc.scalar.activation(nb_, pn[:, :, :NW], ACT.Copy)
                        nc.sync.dma_start(out=ro_l[CV:CV + 1, :], in_=nb_)
                    with tc.tile_pool(name="psO", bufs=2, space="PSUM") as psO:
                        for mc in range(MCV):
                            po = psO.tile([128, NB, 512], F32)
                            for k in range(NCH):
                                for j in range(NB):
                                    nc.tensor.matmul(
                                        po[:, j, :NW],
                                        lhsT=mvt3[:, k, mc * 128:(mc + 1) * 128],
                                        rhs=Wt[:, k, j * NW:(j + 1) * NW],
                                        start=(k == 0), stop=(k == NCH - 1))
                            ob = wkO.tile([128, NB, NW], F32, tag="ob")
                            nc.scalar.activation(ob, po[:, :, :NW], ACT.Copy)
                            nc.sync.dma_start(
                                out=ro_l[mc * 128:(mc + 1) * 128, :], in_=ob)

            if PH < 8:
                raise _Trunc()
            # norm row: tiny AllReduce; value rows: ReduceScatter (CV sharded
            # across cores; host concatenates the 8 normalized slices)
            nc.gpsimd.collective_compute(
                "AllReduce", ALU.add, replica_groups=groups,
                ins=[ro_l[CV:CV + 1, :]], outs=[nrm_g[:]])
            nc.gpsimd.collective_compute(
                "ReduceScatter", ALU.add, replica_groups=groups,
                ins=[ro_l[0:CV, :]], outs=[rs_l[:]])

            # normalize my 64-row slice: out = rs_l / nrm_g
            MCL = CV // ND
            with tc.tile_pool(name="wkG", bufs=2) as wkG, \
                 tc.tile_pool(name="psG", bufs=1, space="PSUM") as psG:
                nrm = wkG.tile([128, NCH], F32, tag="nrm")
                nc.sync.dma_start(
                    out=nrm,
                    in_=nrm_g[:].rearrange("p (m q) -> p q m", q=128))
                rc = wkG.tile([128, NCH], F32, tag="rc")
                nc.vector.reciprocal(rc, nrm)
                rcrow = wkG.tile([1, HW], F32, tag="rcrow")
                part_to_row(scr[5], rcrow[:], rc[:])
                pr = psG.tile([MCL, NB, 512], F32)
                for j in range(NB):
                    nc.tensor.matmul(pr[:, j, :NW], lhsT=ones_r[:, 0:MCL],
                                     rhs=rcrow[:, j * NW:(j + 1) * NW],
                                     start=True, stop=True)
                ch = wkG.tile([MCL, HW], F32, tag="ch")
                nc.sync.dma_start(out=ch, in_=rs_l[:])
                oc = wkG.tile([MCL, NB, NW], F32, tag="oc")
                nc.vector.tensor_mul(
                    oc, ch.rearrange("p (j n) -> p j n", n=NW), pr[:, :, :NW])
                nc.sync.dma_start(out=out_d[:], in_=oc)

        except _Trunc:
            pass
    if not nc.is_finalized():
        nc.finalize()
    return nc


def _host_inputs(mk, qk, mv):
    mkf = np.asarray(mk, np.float32).reshape(CK, THW)
    qkf = np.asarray(qk, np.float32).reshape(CK, HW)
    mvf = np.asarray(mv, np.float32).reshape(CV, THW)
    c = (qkf * qkf).sum(0)
    a = (mkf * mkf).sum(0)
    yv = (np.arange(HW, dtype=np.float32) // W)
    xv = (np.arange(HW, dtype=np.float32) % W)

    q1 = np.empty((65, HW), np.float32)
    q1[0] = c / 8.0
    q1[1:65] = qkf
    q2 = np.empty((67, HW), np.float32)
    q2[0:64] = qkf
    q2[64] = yv * CG
    q2[65] = xv * CG
    q2[66] = -1.0
    q2c = np.empty((68, HW), np.float32)
    q2c[0:64] = qkf
    q2c[64] = yv * CG
    q2c[65] = xv * CG
    q2c[66] = -1.0
    q2c[67] = 0.0

    in_maps = []
    for d in range(ND):
        sl = slice(d * ML, (d + 1) * ML)
        msa = np.zeros((65, ML), np.float32)
        msa[0] = -1.0
        msa[1:65] = mkf[:, sl] / 4.0
        msb = np.zeros((68, ML), np.float32)
        msb[0:64] = mkf[:, sl] / 4.0
        msb[67] = -1.0
        a8t = np.ascontiguousarray(
            (a[sl] / 8.0).reshape(NCH, 128).T.astype(np.float32))
        mvt = np.ascontiguousarray(
            mvf[:, sl].T.reshape(NCH, 128, CV).transpose(1, 0, 2)
            .reshape(128, NCH * CV)).astype(ml_dtypes.bfloat16)
        in_maps.append({
            "msa": msa, "msb": msb, "q1": q1, "q2": q2, "q2c": q2c,
            "a8t": a8t, "mvt": mvt,
        })
    return in_maps


_NC_CACHE = {}


def _get_nc():
    if "nc" not in _NC_CACHE:
        _NC_CACHE["nc"] = _build()
    return _NC_CACHE["nc"]


def assemble(per_core_outs):
    """Concatenate the 8 cores' 64-row CV slices into the full output."""
    out = np.concatenate(
        [np.asarray(o, np.float32) for o in per_core_outs], axis=0)
    return out.reshape(1, CV, H, W)


def kernel(mk, qk, mv):
    from concourse.bass_utils import run_bass_kernel_spmd
    in_maps = _host_inputs(mk, qk, mv)
    nc = _get_nc()
    res = run_bass_kernel_spmd(nc, in_maps, core_ids=list(range(ND)))
    return assemble([res.results[d]["out"] for d in range(ND)])



# revision 15
# speedup vs baseline: 1.2092x; 1.2092x over previous
"""Trainium2 Bass kernel for EvalMemoryReader (retrieval_knn).

Distributed plan (8 NeuronCores, memory axis THW sharded -> 1 frame/core):
  A. stage-1 fp32 matmul (layout A: mem rows x queries) -> per-row argmax
     via DVE max8+find-index -> gaussian center (y,x) per memory row.
  B. stage-2 fp32 matmul (layout B: queries x mem rows) with augmented
     channels folding the gaussian + per-row constants into the contraction
     -> selection scores s_hat.  Per-16 segment maxes -> AllToAll (query
     sharded) -> rank-51-of-segmaxes threshold t + column max via 7 rounds
     of max8/match_replace -> AllGather.  Local survivor counts + 8 smallest
     survivors per query (masked max8) -> AllGather -> exact v50/v51 of the
     global top-50 boundary -> tau = midpoint.
  C. fp32 matmul (layout A) recomputes scores minus column max; mask at tau
     (exact top-50) * exp -> bf16 weights; bf16 readout matmul with a ones
     row appended for the softmax normalizer; AllReduce; normalize.

kernel() takes FULL inputs, shards host-side, runs SPMD on cores 0-7.
"""

import math
import os

import ml_dtypes
import numpy as np

import concourse.bass as bass
import concourse.bacc as bacc
import concourse.mybir as mybir
from concourse.tile import TileContext

ND = 8
CK, CV, T, H, W = 64, 512, 8, 32, 56
HW = H * W              # 1792 queries
THW = T * HW            # 14336 memory locations
ML = THW // ND          # 1792 memory rows per core (exactly one frame)
NCH = HW // 128         # 14 chunks of 128 (query or mem rows)
NB = 4                  # 448-wide free-dim chunks per 1792
NW = HW // NB           # 448
SEG = 16
NSEG = ML // SEG        # 112 segments per core
NQ = HW // ND           # 224 queries per core for threshold extraction
GD = 2.0 * 5.6 * 5.6    # 62.72
CG = math.sqrt(2.0 / GD)
BIG = 1.0e30
NEG = -1.0e30
MCV = CV // 128         # 4 output chunks

F32 = mybir.dt.float32
F32R = mybir.dt.float32r
BF16 = mybir.dt.bfloat16
U32 = mybir.dt.uint32
ALU = mybir.AluOpType
ACT = mybir.ActivationFunctionType
AX = mybir.AxisListType


class _Trunc(Exception):
    pass


def _build():
    nc = bacc.Bacc(num_devices=ND)

    msa_d = nc.dram_tensor("msa", [65, ML], F32, kind="ExternalInput")
    msb_d = nc.dram_tensor("msb", [68, ML], F32, kind="ExternalInput")
    q1_d = nc.dram_tensor("q1", [65, HW], F32, kind="ExternalInput")
    q2_d = nc.dram_tensor("q2", [67, HW], F32, kind="ExternalInput")
    q2c_d = nc.dram_tensor("q2c", [68, HW], F32, kind="ExternalInput")
    a8t_d = nc.dram_tensor("a8t", [128, NCH], F32, kind="ExternalInput")
    mvt_d = nc.dram_tensor("mvt", [128, NCH * CV], BF16, kind="ExternalInput")
    out_d = nc.dram_tensor("out", [CV // ND + 1, HW], F32, kind="ExternalOutput")

    iota16_c = nc.inline_tensor(
        np.broadcast_to(np.arange(16, dtype=np.float32), (128, 16)).copy(),
        name="iota16")
    ones_1x128_c = nc.inline_tensor(np.ones((1, 128), np.float32), name="o1x128")
    ones_128x1_c = nc.inline_tensor(
        np.ones((128, 1), np.float32).astype(ml_dtypes.bfloat16), name="o128x1")
    thr56_c = nc.inline_tensor(
        np.broadcast_to(np.arange(1, H, dtype=np.float32) * W, (128, H - 1))
        .copy(), name="thr56")

    # collective bounce buffers
    segmax_l = nc.dram_tensor("segmax_l", [HW, NSEG], F32)
    segmax_x = nc.dram_tensor("segmax_x", [HW, NSEG], F32)
    tstats_l = nc.dram_tensor("tstats_l", [NQ, 1], F32)
    tstats_g = nc.dram_tensor("tstats_g", [HW, 1], F32, addr_space="Shared")
    stats_l = nc.dram_tensor("stats_l", [HW, 9], F32)
    stats_g = nc.dram_tensor("stats_g", [ND, HW, 9], F32, addr_space="Shared")
    scr = [nc.dram_tensor(f"scr{i}", [HW], F32) for i in range(6)]
    # readout rows interleaved in groups of 65 per core: rows 65d..65d+63 are
    # value rows 64d..64d+63, row 65d+64 is a copy of the local norm row, so a
    # single ReduceScatter delivers each core its value slice + global norm.
    ro_l = nc.dram_tensor("ro_l", [(CV // ND + 1) * ND, HW], F32)

    groups = [list(range(ND))]

    from contextlib import ExitStack
    with TileContext(nc) as tc, ExitStack() as es:
        try:
            POOL_E = mybir.EngineType.Pool
            cpool = es.enter_context(tc.tile_pool(name="consts", bufs=1))
            def cload(ap, name):
                return cpool.tile_from(ap, force_copy=True, name=name,
                                       forced_dma_engine=POOL_E)
            msa = cload(msa_d[:], "msa_t")
            msb = cload(msb_d[:], "msb_t")
            q1 = cload(q1_d[:], "q1_t")
            q2 = cload(q2_d[:], "q2_t")
            q2c = cload(q2c_d[:], "q2c_t")
            a8t = cload(a8t_d[:], "a8t_t")
            iota16 = cload(iota16_c[:], "iota16_t")
            ones_cb = cload(ones_128x1_c[:], "ones_cb_t")
            thr56 = cload(thr56_c[:], "thr56_t")

            spool = es.enter_context(tc.tile_pool(name="smalls", bufs=1))
            ycg_t = spool.tile([128, NCH], F32)
            xcg_t = spool.tile([128, NCH], F32)
            alp_t = spool.tile([128, NCH], F32)
            t_all = spool.tile([128, NCH], F32)
            cnt_t = spool.tile([128, NCH], F32)
            mins_t = spool.tile([128, NCH, 8], F32)
            tauc_t = spool.tile([128, NCH], F32)

            # ---------------- phase A: argmax per memory row ----------------
            with tc.tile_pool(name="psA", bufs=2, space="PSUM") as psA, \
                 tc.tile_pool(name="wkA", bufs=3) as wkA:
                for m in range(NCH):
                    ps = psA.tile([128, NB, 512], F32)
                    for j in range(NB):
                        nc.tensor.matmul(
                            ps[:, j, :NW],
                            lhsT=msa[:, m * 128:(m + 1) * 128].bitcast(F32R),
                            rhs=q1[:, j * NW:(j + 1) * NW].bitcast(F32R),
                            start=True, stop=True)
                    u = wkA.tile([128, HW], F32, tag="u")
                    nc.scalar.activation(
                        u.rearrange("p (j n) -> p j n", n=NW), ps[:, :, :NW], ACT.Copy)
                    m8 = wkA.tile([128, 8], F32, tag="m8")
                    i8 = wkA.tile([128, 8], U32, tag="i8")
                    nc.vector.max(m8, u)
                    nc.vector.max_index(i8, m8, u)
                    idxf = wkA.tile([128, 1], F32, tag="idxf")
                    nc.vector.tensor_copy(idxf, i8[:, 0:1])
                    xm = wkA.tile([128, 1], F32, tag="xm")
                    ym = wkA.tile([128, 1], F32, tag="ym")
                    jnk = wkA.tile([128, H - 1], F32, tag="jnk")
                    # y = #{k in 1..31 : k*W <= idx} = idx // W  (no mod/div on DVE)
                    nc.vector.tensor_scalar(jnk, thr56, idxf, None, op0=ALU.is_le,
                                            op1=ALU.add, accum_out=ym)
                    # x = idx - W*y
                    nc.vector.scalar_tensor_tensor(xm, ym, -float(W), idxf,
                                                   op0=ALU.mult, op1=ALU.add)
                    # scales/squares on ACT: keeps them off the DVE stream
                    # that the max8/max_index chain is saturating
                    nc.scalar.activation(ycg_t[:, m:m + 1], ym, ACT.Copy,
                                         scale=CG)
                    nc.scalar.activation(xcg_t[:, m:m + 1], xm, ACT.Copy,
                                         scale=CG)
                    # alpha = a8 + (y^2 + x^2)/GD ; note (y*cg)^2+(x*cg)^2 = 2(y^2+x^2)/GD
                    ysq = wkA.tile([128, 1], F32, tag="ysq")
                    nc.vector.tensor_mul(ysq, ycg_t[:, m:m + 1], ycg_t[:, m:m + 1])
                    xsq = wkA.tile([128, 1], F32, tag="xsq")
                    nc.vector.tensor_mul(xsq, xcg_t[:, m:m + 1], xcg_t[:, m:m + 1])
                    ssum = wkA.tile([128, 1], F32, tag="ssum")
                    nc.vector.tensor_add(ssum, ysq, xsq)
                    hsum = wkA.tile([128, 1], F32, tag="hsum")
                    nc.scalar.activation(hsum, ssum, ACT.Copy, scale=0.5)
                    nc.vector.tensor_add(alp_t[:, m:m + 1], hsum, a8t[:, m:m + 1])
                    # stream this chunk's gaussian channels into msb now so
                    # stage-2 matmuls can start before the argmax loop finishes
                    for row, srct in ((64, ycg_t), (65, xcg_t), (66, alp_t)):
                        nc.sync.dma_start(
                            out=msb[row:row + 1, m * 128:(m + 1) * 128],
                            in_=srct[:, m:m + 1])

            # scatter per-row channels into msb rows 64..66 (partition -> free),
            # bounced through DRAM because SBUF<->SBUF transpose APs don't balance
            def part_to_row(scratch, row_ap, tile_ap):
                nc.sync.dma_start(
                    out=scratch[:].rearrange("(m q) -> q m", q=128), in_=tile_ap)
                nc.sync.dma_start(out=row_ap, in_=scratch[:])


            PH = int(os.environ.get("KPHASE", "99"))
            if PH < 2:
                raise _Trunc()
            # ---------------- phase B: scores + top-50 boundary ----------------
            with tc.tile_pool(name="sBpool", bufs=1) as sBpool:
                sB = sBpool.tile([128, NCH, ML], F32)
                with tc.tile_pool(name="psB", bufs=2, space="PSUM") as psB, \
                     tc.tile_pool(name="wkB", bufs=3) as wkB:
                    for m in range(NCH):
                        ps = psB.tile([128, NB, 512], F32)
                        for j in range(NB):
                            nc.tensor.matmul(
                                ps[:, j, :NW],
                                lhsT=q2[:, m * 128:(m + 1) * 128].bitcast(F32R),
                                rhs=msb[0:67, j * NW:(j + 1) * NW].bitcast(F32R),
                                start=True, stop=True)
                        nc.scalar.activation(
                            sB[:, m, :].rearrange("p (j n) -> p j n", n=NW),
                            ps[:, :, :NW], ACT.Copy)
                        seg = wkB.tile([128, NSEG], F32, tag="seg")
                        nc.vector.tensor_reduce(
                            seg, sB[:, m, :].rearrange("p (s k) -> p s k", k=SEG),
                            axis=AX.X, op=ALU.max)
                        nc.sync.dma_start(out=segmax_l[m * 128:(m + 1) * 128, :],
                                          in_=seg[:])

                if PH < 3:
                    raise _Trunc()
                nc.gpsimd.collective_compute(
                    "AllToAll", ALU.bypass, replica_groups=groups,
                    ins=[segmax_l[:]], outs=[segmax_x[:]])

                # rank-51 of global segmaxes, for my 224 queries
                with tc.tile_pool(name="wkT", bufs=2) as wkT:
                    for ci, (p0, pc) in enumerate(((0, 128), (128, 96))):
                        ext = wkT.tile([pc, ND * NSEG], F32, tag=f"ext{ci}")
                        src = segmax_x[:].rearrange("(d p) s -> p d s", d=ND)
                        nc.sync.dma_start(
                            out=ext.rearrange("p (d s) -> p d s", d=ND),
                            in_=src[p0:p0 + pc, :, :])
                        m8 = wkT.tile([pc, 8], F32, tag=f"m8{ci}")
                        for r in range(7):
                            nc.vector.max(m8, ext)
                            if r < 6:
                                nc.vector.match_replace(ext, m8, ext, NEG)
                        nc.sync.dma_start(out=tstats_l[p0:p0 + pc, 0:1],
                                          in_=m8[:, 2:3])

                nc.gpsimd.collective_compute(
                    "AllGather", ALU.bypass, replica_groups=groups,
                    ins=[tstats_l[:]], outs=[tstats_g[:]])

                nc.sync.dma_start(
                    out=t_all[:],
                    in_=tstats_g[:, 0:1].rearrange("(m q) s -> q (m s)", q=128))

                if PH < 4:
                    raise _Trunc()
                # local survivor count + 8 smallest survivors per query
                # (count/maskneg alternate DVE and Pool so the two element
                # scans run on both vector engines concurrently)
                with tc.tile_pool(name="wkC", bufs=3) as wkC:
                    for m in range(NCH):
                        eng = nc.vector if m % 2 == 0 else nc.gpsimd
                        msk = wkC.tile([128, ML], F32, tag="msk")
                        eng.tensor_scalar(
                            msk, sB[:, m, :], t_all[:, m:m + 1], None,
                            op0=ALU.is_lt, op1=ALU.add,
                            accum_out=cnt_t[:, m:m + 1])
                        r = wkC.tile([128, ML], F32, tag="r")
                        eng.scalar_tensor_tensor(
                            r, msk, -BIG, sB[:, m, :], op0=ALU.mult, op1=ALU.subtract)
                        mn = wkC.tile([128, 8], F32, tag="mn")
                        nc.vector.max(mn, r)
                        nc.vector.tensor_scalar(mins_t[:, m, :], mn, -1.0, None,
                                                op0=ALU.mult)
                        nc.sync.dma_start(out=stats_l[m * 128:(m + 1) * 128, 0:1],
                                          in_=cnt_t[:, m:m + 1])
                        nc.sync.dma_start(out=stats_l[m * 128:(m + 1) * 128, 1:9],
                                          in_=mins_t[:, m, :])

            nc.gpsimd.collective_compute(
                "AllGather", ALU.bypass, replica_groups=groups,
                ins=[stats_l[:]], outs=[stats_g[:]])

            if PH < 5:
                raise _Trunc()
            # global boundary: v50/v51 -> tau, redundant on every core
            with tc.tile_pool(name="wkD", bufs=3) as wkD:
                for m in range(NCH):
                    mins64 = wkD.tile([128, ND, 8], F32, tag="m64")
                    nc.sync.dma_start(
                        out=mins64,
                        in_=stats_g[:, m * 128:(m + 1) * 128, 1:9]
                        .rearrange("d p s -> p d s"))
                    cnt8 = wkD.tile([128, ND], F32, tag="c8")
                    nc.sync.dma_start(
                        out=cnt8,
                        in_=stats_g[:, m * 128:(m + 1) * 128, 0:1]
                        .rearrange("d p s -> p (d s)"))
                    cl = wkD.tile([128, 1], F32, tag="cl")
                    nc.vector.tensor_reduce(cl, cnt8, axis=AX.X, op=ALU.add)
                    e = wkD.tile([128, 1], F32, tag="e")
                    # e = (THW - cnt_lt_total) - 50
                    nc.vector.tensor_scalar(e, cl, -1.0, float(THW - 50),
                                            op0=ALU.mult, op1=ALU.add)
                    neg64 = wkD.tile([128, ND * 8], F32, tag="n64")
                    nc.vector.tensor_scalar(
                        neg64, mins64.rearrange("p d s -> p (d s)"), -1.0, None,
                        op0=ALU.mult)
                    asc16 = wkD.tile([128, 16], F32, tag="a16")
                    a8a = wkD.tile([128, 8], F32, tag="a8a")
                    nc.vector.max(a8a, neg64)
                    nc.vector.tensor_scalar(asc16[:, 0:8], a8a, -1.0, None,
                                            op0=ALU.mult)
                    nc.vector.match_replace(neg64, a8a, neg64, NEG)
                    nc.vector.max(a8a, neg64)
                    nc.vector.tensor_scalar(asc16[:, 8:16], a8a, -1.0, None,
                                            op0=ALU.mult)
                    v50 = wkD.tile([128, 1], F32, tag="v50")
                    v51 = wkD.tile([128, 1], F32, tag="v51")
                    mk16 = wkD.tile([128, 16], F32, tag="mk16")
                    junk = wkD.tile([128, 16], F32, tag="junk")
                    nc.vector.tensor_scalar(mk16, iota16, e, None, op0=ALU.is_equal)
                    nc.vector.scalar_tensor_tensor(
                        junk, asc16, 1.0, mk16, op0=ALU.mult, op1=ALU.mult,
                        accum_out=v50)
                    em1 = wkD.tile([128, 1], F32, tag="em1")
                    nc.vector.tensor_scalar(em1, e, 1.0, None, op0=ALU.subtract)
                    nc.vector.tensor_scalar(mk16, iota16, em1, None, op0=ALU.is_equal)
                    nc.vector.scalar_tensor_tensor(
                        junk, asc16, 1.0, mk16, op0=ALU.mult, op1=ALU.mult,
                        accum_out=v51)
                    tau = wkD.tile([128, 1], F32, tag="tau")
                    nc.vector.tensor_add(tau, v50, v51)
                    nc.vector.tensor_scalar(tauc_t[:, m:m + 1], tau, 0.5, None,
                                            op0=ALU.mult)

            # tau (absolute) becomes q2c channel 67: psC = s_hat - tau, so the
            # top-50 mask is just sign(psC) and exp(psC) is range-safe.
            part_to_row(scr[3], q2c[67:68, :], tauc_t[:])

            if PH < 6:
                raise _Trunc()
            # ---------------- phase C: weights + readout ----------------
            with tc.tile_pool(name="Wpool", bufs=1) as Wpool:
                Wt = Wpool.tile([128, NCH, ML], BF16)
                with tc.tile_pool(name="psC", bufs=2, space="PSUM") as psC, \
                     tc.tile_pool(name="wkF", bufs=3) as wkF:
                    for k in range(NCH):
                        ps = psC.tile([128, NB, 512], F32)
                        for j in range(NB):
                            nc.tensor.matmul(
                                ps[:, j, :NW],
                                lhsT=msb[:, k * 128:(k + 1) * 128].bitcast(F32R),
                                rhs=q2c[:, j * NW:(j + 1) * NW].bitcast(F32R),
                                start=True, stop=True)
                        # psC = s - tau: z = min(ps*BIG, ps) maps rejected
                        # entries (ps<0) to -huge so exp(z) = masked weight
                        eng = nc.vector if k % 2 == 0 else nc.gpsimd
                        z = wkF.tile([128, NB, NW], F32, tag="z")
                        eng.scalar_tensor_tensor(
                            z, ps[:, :, :NW], BIG, ps[:, :, :NW],
                            op0=ALU.mult, op1=ALU.min)
                        nc.scalar.activation(
                            Wt[:, k, :].rearrange("p (j n) -> p j n", n=NW),
                            z, ACT.Exp)

                if PH < 7:
                    raise _Trunc()
                with tc.tile_pool(name="mvpool", bufs=1) as mvpool, \
                     tc.tile_pool(name="wkO", bufs=2) as wkO:
                    mvt = mvpool.tile_from(mvt_d[:], force_copy=True,
                                           forced_dma_engine=POOL_E)
                    mvt3 = mvt.rearrange("p (k c) -> p k c", c=CV)
                    # norm row first in a short-lived PSUM pool so the value
                    # chunks below can double-buffer their PSUM tiles
                    with tc.tile_pool(name="psN", bufs=1, space="PSUM") as psN:
                        pn = psN.tile([1, NB, 512], F32)
                        for k in range(NCH):
                            for j in range(NB):
                                nc.tensor.matmul(
                                    pn[:, j, :NW], lhsT=ones_cb,
                                    rhs=Wt[:, k, j * NW:(j + 1) * NW],
                                    start=(k == 0), stop=(k == NCH - 1))
                        nb_ = wkO.tile([1, NB, NW], F32, tag="nb")
                        nc.scalar.activation(nb_, pn[:, :, :NW], ACT.Copy)
                        for d in range(ND):
                            nc.sync.dma_start(
                                out=ro_l[d * 65 + 64:d * 65 + 65, :]
                                .rearrange("r (j n) -> r j n", n=NW),
                                in_=nb_)
                    with tc.tile_pool(name="psO", bufs=2, space="PSUM") as psO:
                        for mc in range(MCV):
                            po = psO.tile([128, NB, 512], F32)
                            for k in range(NCH):
                                for j in range(NB):
                                    nc.tensor.matmul(
                                        po[:, j, :NW],
                                        lhsT=mvt3[:, k, mc * 128:(mc + 1) * 128],
                                        rhs=Wt[:, k, j * NW:(j + 1) * NW],
                                        start=(k == 0), stop=(k == NCH - 1))
                            ob = wkO.tile([128, NB, NW], F32, tag="ob")
                            nc.scalar.activation(ob, po[:, :, :NW], ACT.Copy)
                            # value rows v=64d+i land at ro row 65d+i
                            for h in range(2):
                                d = 2 * mc + h
                                nc.sync.dma_start(
                                    out=ro_l[d * 65:d * 65 + 64, :]
                                    .rearrange("r (j n) -> r j n", n=NW),
                                    in_=ob[h * 64:(h + 1) * 64])

            if PH < 8:
                raise _Trunc()
            # one ReduceScatter delivers 64 summed value rows + the summed
            # norm row to each core; the host does the division.
            nc.gpsimd.collective_compute(
                "ReduceScatter", ALU.add, replica_groups=groups,
                ins=[ro_l[:]], outs=[out_d[:]])

        except _Trunc:
            pass
    if not nc.is_finalized():
        nc.finalize()
    return nc


def _host_inputs(mk, qk, mv):
    mkf = np.asarray(mk, np.float32).reshape(CK, THW)
    qkf = np.asarray(qk, np.float32).reshape(CK, HW)
    mvf = np.asarray(mv, np.float32).reshape(CV, THW)
    c = (qkf * qkf).sum(0)
    a = (mkf * mkf).sum(0)
    yv = (np.arange(HW, dtype=np.float32) // W)
    xv = (np.arange(HW, dtype=np.float32) % W)

    q1 = np.empty((65, HW), np.float32)
    q1[0] = c / 8.0
    q1[1:65] = qkf
    q2 = np.empty((67, HW), np.float32)
    q2[0:64] = qkf
    q2[64] = yv * CG
    q2[65] = xv * CG
    q2[66] = -1.0
    q2c = np.empty((68, HW), np.float32)
    q2c[0:64] = qkf
    q2c[64] = yv * CG
    q2c[65] = xv * CG
    q2c[66] = -1.0
    q2c[67] = 0.0

    in_maps = []
    for d in range(ND):
        sl = slice(d * ML, (d + 1) * ML)
        msa = np.zeros((65, ML), np.float32)
        msa[0] = -1.0
        msa[1:65] = mkf[:, sl] / 4.0
        msb = np.zeros((68, ML), np.float32)
        msb[0:64] = mkf[:, sl] / 4.0
        msb[67] = -1.0
        a8t = np.ascontiguousarray(
            (a[sl] / 8.0).reshape(NCH, 128).T.astype(np.float32))
        mvt = np.ascontiguousarray(
            mvf[:, sl].T.reshape(NCH, 128, CV).transpose(1, 0, 2)
            .reshape(128, NCH * CV)).astype(ml_dtypes.bfloat16)
        in_maps.append({
            "msa": msa, "msb": msb, "q1": q1, "q2": q2, "q2c": q2c,
            "a8t": a8t, "mvt": mvt,
        })
    return in_maps


_NC_CACHE = {}


def _get_nc():
    if "nc" not in _NC_CACHE:
        _NC_CACHE["nc"] = _build()
    return _NC_CACHE["nc"]


def assemble(per_core_outs):
    """Each core returns [65, HW]: 64 summed value rows + the summed norm
    row. Normalize host-side and concatenate the 8 slices."""
    parts = []
    for o in per_core_outs:
        o = np.asarray(o, np.float32)
        parts.append(o[0:CV // ND] / o[CV // ND:CV // ND + 1])
    return np.concatenate(parts, axis=0).reshape(1, CV, H, W)


def kernel(mk, qk, mv):
    from concourse.bass_utils import run_bass_kernel_spmd
    in_maps = _host_inputs(mk, qk, mv)
    nc = _get_nc()
    res = run_bass_kernel_spmd(nc, in_maps, core_ids=list(range(ND)))
    return assemble([res.results[d]["out"] for d in range(ND)])

